# revision 2
# baseline (speedup 1.0000x reference)
"""TF-IDF document model (histogram_binning) on 8 TRN2 NeuronCores — v2.

Per core, 64 batch rows. Radix one-hot histogram: v = hi*393 + lo.
Engine-balanced build pipeline per row:
  - DVE: one batched A2 one-hot build (TensorTensor is_equal, 2x mode,
    h-major layout [h*CH+c]), 4 B-chunk builds (tensor_scalar, 4x mode),
    and the stt (T = C*idf2 fp16 + per-row accum for n).
  - Pool (gpsimd): ~3 B-chunk builds per row (tensor_scalar is_equal).
  - ACT: ~1 B-chunk build per row via 2-pass integer-delta
    (Square then Relu: onehot = relu(1 - 1024*((iota-lo)/32)^2)),
    per-group reciprocal + rb copy, and the final per-row scale to fp32.
  - PE: 8 fp16 accumulate matmuls per row (lhsT = strided h-major view).
Output written as [128, 64*393] per core; host slices/concats.
"""
import numpy as np

import concourse.bacc as bacc
import concourse.mybir as mybir
from concourse import bass_utils
from concourse.tile import TileContext

B, S, V = 512, 1024, 50257
NC = 8
BL = B // NC          # 64 rows per core
HI, LO = 128, 393     # radix split: v = hi*LO + lo
VP = HI * LO          # 50304 padded vocab
CH = S // 128         # 8 token chunks per row
GROUP = 8             # rows per normalization group

# per-row B-chunk assignment: chunk index -> engine
# 8 chunks: 4 on DVE, 3 on Pool, 1 on ACT (rows 0-5 of each group)
#           4 on DVE, 2 on Pool, 2 on ACT (rows 6-7)
_cache = {}


def _build(repeat: int = 0, strided_lhsT: bool = True):
    nc = bacc.Bacc(
        "TRN2",
        target_bir_lowering=False,
        debug=False,
        enable_asserts=False,
        num_devices=NC,
    )
    hif_t = nc.dram_tensor("hif", [128, BL * CH], mybir.dt.float16, kind="ExternalInput")
    lof_t = nc.dram_tensor("lof", [128, BL * CH], mybir.dt.float32, kind="ExternalInput")
    lof32_t = nc.dram_tensor("lof32", [128, BL * CH], mybir.dt.float32, kind="ExternalInput")
    idf2_t = nc.dram_tensor("idf2", [HI, LO], mybir.dt.float32, kind="ExternalInput")
    iotaA_t = nc.dram_tensor("iotaA", [128, HI * CH], mybir.dt.float16, kind="ExternalInput")
    iota_t = nc.dram_tensor("iota", [128, LO], mybir.dt.float16, kind="ExternalInput")
    iota32_t = nc.dram_tensor("iota32", [128, LO], mybir.dt.float16, kind="ExternalInput")
    onesc_t = nc.dram_tensor("onesc", [128, 1], mybir.dt.float32, kind="ExternalInput")
    onesr_t = nc.dram_tensor("onesr", [1, 128], mybir.dt.float32, kind="ExternalInput")
    out_t = nc.dram_tensor("out", [128, BL * LO], mybir.dt.float32, kind="ExternalOutput")
    ovg = out_t.ap().rearrange("p (g c) -> g p c", g=BL // GROUP)

    AF = mybir.ActivationFunctionType
    OP = mybir.AluOpType

    with TileContext(nc) as tc:
        with (
            tc.tile_pool(name="const", bufs=1) as cpool,
            tc.tile_pool(name="a2", bufs=3) as a2pool,
            tc.tile_pool(name="b2", bufs=3) as b2pool,
            tc.tile_pool(name="sq", bufs=3) as sqpool,
            tc.tile_pool(name="w", bufs=6) as wpool,
            tc.tile_pool(name="tt", bufs=3) as tpool,
            tc.tile_pool(name="og", bufs=2) as opool,
            tc.tile_pool(name="ps", bufs=4, space="PSUM") as pspool,
            tc.tile_pool(name="ps2", bufs=2, space="PSUM") as ps2pool,
        ):
            idf2 = cpool.tile([HI, LO], mybir.dt.float32, tag="idf2")
            nc.sync.dma_start(out=idf2[:], in_=idf2_t.ap())
            iotaA = cpool.tile([128, HI * CH], mybir.dt.float16, tag="iotaA")
            nc.sync.dma_start(out=iotaA[:], in_=iotaA_t.ap())
            iota = cpool.tile([128, LO], mybir.dt.float16, tag="iota")
            nc.sync.dma_start(out=iota[:], in_=iota_t.ap())
            iota32 = cpool.tile([128, LO], mybir.dt.float16, tag="iota32")
            nc.sync.dma_start(out=iota32[:], in_=iota32_t.ap())
            onesc = cpool.tile([128, 1], mybir.dt.float32, tag="onesc")
            nc.sync.dma_start(out=onesc[:], in_=onesc_t.ap())
            onesr = cpool.tile([1, 128], mybir.dt.float32, tag="onesr")
            nc.sync.dma_start(out=onesr[:], in_=onesr_t.ap())
            hif = cpool.tile([128, BL * CH], mybir.dt.float16, tag="hif")
            nc.sync.dma_start(out=hif[:], in_=hif_t.ap())
            lof = cpool.tile([128, BL * CH], mybir.dt.float32, tag="lof")
            nc.sync.dma_start(out=lof[:], in_=lof_t.ap())
            lof32 = cpool.tile([128, BL * CH], mybir.dt.float32, tag="lof32")
            nc.sync.dma_start(out=lof32[:], in_=lof32_t.ap())

            def main_body(_iv=None):
              tail = []  # deferred normalization tails: [g, Tg, rb, OUT, next_r]

              def emit_tail_step():
                  # emit one deferred scale (and the DMA after the last one)
                  if not tail:
                      return
                  st = tail[0]
                  gq, Tgq, rbq, OUT, r = st
                  if OUT is None:
                      OUT = st[3] = opool.tile(
                          [128, GROUP * LO], mybir.dt.float32, tag="OUT",
                          name=f"OUT_{gq}")
                  nc.scalar.activation(
                      out=OUT[:, r * LO:(r + 1) * LO],
                      in_=Tgq[:, r * LO:(r + 1) * LO],
                      func=AF.Copy, scale=rbq[:, r:r + 1])
                  st[4] += 1
                  if st[4] == GROUP:
                      nc.sync.dma_start(out=ovg[gq], in_=OUT[:])
                      tail.pop(0)

              for g in range(BL // GROUP):
                nsums = wpool.tile([128, GROUP], mybir.dt.float32, tag="nsums")
                Tg = tpool.tile([128, GROUP * LO], mybir.dt.float16, tag="Tg")
                pending = []

                def emit_stt(rr, CC):
                    nc.vector.scalar_tensor_tensor(
                        out=Tg[:, rr * LO:(rr + 1) * LO],
                        in0=CC[:], scalar=1.0, in1=idf2[:],
                        op0=OP.mult, op1=OP.mult,
                        accum_out=nsums[:, rr:rr + 1],
                    )

                for r in range(GROUP):
                    row = g * GROUP + r
                    col0 = row * CH
                    # batched A2 build on DVE (h-major [h*CH+c])
                    A2 = a2pool.tile([128, HI * CH], mybir.dt.float16, tag="A2")
                    nc.vector.tensor_tensor(
                        out=A2[:], in0=iotaA[:],
                        in1=hif[:, col0:col0 + CH].unsqueeze(1)
                            .broadcast_to([128, HI, CH]),
                        op=OP.is_equal)
                    # B2 chunk builds split across DVE / Pool / ACT
                    B2 = b2pool.tile([128, CH * LO], mybir.dt.float16, tag="B2")
                    n_act = 1 if r < 6 else 2
                    for c in range(CH):
                        dst = B2[:, c * LO:(c + 1) * LO]
                        scol = lof[:, col0 + c:col0 + c + 1]
                        if c < 4:
                            nc.vector.tensor_scalar(
                                out=dst, in0=iota[:], scalar1=scol,
                                scalar2=None, op0=OP.is_equal)
                        elif c < 8 - n_act:
                            nc.gpsimd.tensor_scalar(
                                out=dst, in0=iota[:], scalar1=scol,
                                scalar2=None, op0=OP.is_equal)
                        else:
                            sq = sqpool.tile([128, LO], mybir.dt.float16, tag="sq")
                            nc.scalar.activation(
                                out=sq[:], in_=iota32[:], func=AF.Square,
                                bias=lof32[:, col0 + c:col0 + c + 1], scale=1.0)
                            nc.scalar.activation(
                                out=dst, in_=sq[:], func=AF.Relu,
                                bias=1.0, scale=-1024.0)
                    if len(pending) >= 2:
                        emit_stt(*pending.pop(0))
                    emit_tail_step()
                    # 8 fp16 accumulate matmuls
                    C = pspool.tile([HI, LO], mybir.dt.float32, tag="C")
                    if strided_lhsT:
                        av = A2[:].rearrange("p (h c) -> p c h", c=CH)
                        for c in range(CH):
                            nc.tensor.matmul(
                                out=C[:], lhsT=av[:, c, :],
                                rhs=B2[:, c * LO:(c + 1) * LO],
                                start=(c == 0), stop=(c == CH - 1))
                    else:
                        raise NotImplementedError
                    pending.append((r, C))

                while pending:
                    emit_stt(*pending.pop(0))
                # group normalization chain
                n_ps = ps2pool.tile([1, GROUP], mybir.dt.float32, tag="nps")
                nc.tensor.matmul(out=n_ps[:], lhsT=onesc[:], rhs=nsums[:],
                                 start=True, stop=True)
                recip = wpool.tile([1, GROUP], mybir.dt.float32, tag="recip")
                nc.vector.reciprocal(out=recip[:], in_=n_ps[:])
                rb_ps = ps2pool.tile([128, GROUP], mybir.dt.float32, tag="rbps")
                nc.tensor.matmul(out=rb_ps[:], lhsT=onesr[:], rhs=recip[:],
                                 start=True, stop=True)
                rb = wpool.tile([128, GROUP], mybir.dt.float32, tag="rb")
                nc.scalar.activation(out=rb[:], in_=rb_ps[:], func=AF.Copy)
                tail.append([g, Tg, rb, None, 0])

              while tail:
                  emit_tail_step()

            if repeat:
                tc.For_i_unrolled(0, repeat, 1, main_body, max_unroll=1)
            else:
                main_body()
    nc.compile()
    return nc


def _get_nc():
    if "nc" not in _cache:
        _cache["nc"] = _build()
    return _cache["nc"]


def _host_inputs(x: np.ndarray, idf: np.ndarray):
    idf_pad = np.zeros(VP, dtype=np.float32)
    idf_pad[:V] = np.asarray(idf, dtype=np.float32)
    idf2 = idf_pad.reshape(HI, LO)
    iota = np.broadcast_to(np.arange(LO, dtype=np.float16), (128, LO)).copy()
    iota32 = np.broadcast_to((np.arange(LO) / 32.0).astype(np.float16), (128, LO)).copy()
    iotaA = np.broadcast_to(
        (np.arange(HI, dtype=np.float16)[:, None] * np.ones((1, CH), np.float16))
        .reshape(1, HI * CH), (128, HI * CH)).copy()
    onesc = np.ones((128, 1), dtype=np.float32)
    onesr = np.ones((1, 128), dtype=np.float32)

    xi = np.asarray(x, dtype=np.int32)
    hi_all = (xi // LO).astype(np.float16)
    lo_all = (xi % LO).astype(np.float32)
    in_maps = []
    for k in range(NC):
        def lay(a, dt):
            ac = a[k * BL:(k + 1) * BL]
            return np.ascontiguousarray(
                ac.reshape(BL, CH, 128).transpose(2, 0, 1).reshape(128, BL * CH)
            ).astype(dt)
        in_maps.append({
            "hif": lay(hi_all, np.float16),
            "lof": lay(lo_all, np.float32),
            "lof32": -(lay(lo_all, np.float32) / 32.0),
            "idf2": idf2, "iotaA": iotaA, "iota": iota, "iota32": iota32,
            "onesc": onesc, "onesr": onesr,
        })
    return in_maps


def kernel(x: np.ndarray, idf: np.ndarray) -> np.ndarray:
    nc = _get_nc()
    in_maps = _host_inputs(x, idf)
    res = bass_utils.run_bass_kernel_spmd(nc, in_maps, core_ids=list(range(NC)))
    outs = []
    for r in res.results:
        a = r["out"].reshape(128, BL, LO).transpose(1, 0, 2).reshape(BL, VP)
        outs.append(a[:, :V])
    return np.concatenate(outs, axis=0)
